# revision 34
# baseline (speedup 1.0000x reference)
"""EucNormLoss Trainium2 kernel (8-core SPMD), v3.

loss = mean_i( sum_j d(i,j)*[l_i==l_j] / #{j: l_j==l_i} ),
d(i,j) = sqrt(2 - 2*fn_i.fn_j) on L2-normalized rows.

Only same-class pairs matter and only the per-slot TOTAL is needed,
so the host sorts classes by size, snake-deals them to the 8 cores,
normalizes rows in fp32, scales by s = 1-2^-7 (which bounds every
uploaded bf16 row norm strictly below 1, so 2 - 2*<b_i,b_j> > 0
always and the on-device sqrt can never see a negative argument --
no diagonal-knockout matmul needed), casts to bf16 and uploads the
TRANSPOSED per-core feature matrix featT [128, nslots*256].

Device work, per pair of slots (pair-uniform width W, so one SPMD
program serves all 8 cores and one 3D-AP ACT call serves 2 slots):
  * 2 Gram matmuls per slot into one PSUM bank, exploiting symmetry:
    with row tiles r0/r1, compute r0 x cols[0:W] and r1 x cols[128:W];
    sum(full) = sum(r0 block) + sum(cols[128:W] of both blocks).
  * ONE ACT sqrt per pair straight out of PSUM (scale=-2, bias=+2),
    3D AP over the two 512-f32-strided slot regions, writing bf16 dq.
  * TWO DVE 3D row-reductions per pair (cols [0,W) -> acc_a and
    [128, 2W-128) -> acc_b) realize the symmetric double-count.
A final weighted reduce + 1-col matmul collapses to the core partial.
Host subtracts, exactly: sqrt(2) per zero-padded pair cell, and the
per-row diagonal mass sqrt(2 - 2*||b_i||^2) computed from the very
bf16 data it uploads (the reference's diagonal contribution is 0).
All input DMA goes through the single SP HWDGE queue in order, so
chunk 0 lands ~1us after issue instead of round-robining with the
other chunks.
"""

import sys

import numpy as np

for _p in ("/opt/trn_rl_repo",):
    if _p not in sys.path:
        sys.path.insert(0, _p)

import ml_dtypes
from contextlib import ExitStack

import concourse.bass as bass
import concourse.bacc as bacc
import concourse.tile as tile
from concourse import mybir
from concourse.bass_utils import run_bass_kernel_spmd
from concourse.masks import make_identity

N_CORES = 8
P = 128          # partitions / feature dim
SLOT = 256       # row capacity per class slot (2 x 128)
SCL = 1.0 - 2.0 ** -7  # row pre-scale: keeps every bf16 row norm < 1
BANK = 512       # PSUM bank width in f32

F32 = mybir.dt.float32
BF16 = mybir.dt.bfloat16


def _bcast_rows(ap: bass.AP, n: int) -> bass.AP:
    return bass.AP(tensor=ap.tensor, offset=ap.offset,
                   ap=[[0, n]] + list(ap.ap[1:]))


def _ap3(t, off: int, stride: int, n: int, width: int) -> bass.AP:
    """[P, n, width] view of tile t at column offset off with the given
    free-dim stride between the n segments."""
    base = t[:, off:] if off else t[:, :]
    return bass.AP(tensor=base.tensor, offset=base.offset,
                   ap=[list(base.ap[0]), [stride, n], [1, width]])


def _groups(nslots):
    """Group sizes: small groups first (low latency to first ACT),
    bigger later (amortize per-instruction overheads)."""
    sizes, rem = [], nslots
    while rem:
        cap = 2 if len(sizes) < 3 else (3 if len(sizes) < 4 else 4)
        take = min(rem, cap)
        sizes.append(take)
        rem -= take
    out, at = [], 0
    for sz in sizes:
        out.append((at, at + sz))
        at += sz
    return out


def _build_program(widths):
    nslots = len(widths)
    cols = nslots * SLOT
    pairs = _groups(nslots)

    F16 = mybir.dt.float16
    nc = bacc.Bacc(None, target_bir_lowering=False)
    ft_d = nc.declare_dram_parameter("feat_t", [P, cols], BF16, isOutput=False)
    out_d = nc.declare_dram_parameter("out", [P, 2 * nslots], F16,
                                      isOutput=True)

    with ExitStack() as ctx:
        tc = ctx.enter_context(tile.TileContext(nc))
        consts = ctx.enter_context(tc.tile_pool(name="consts", bufs=1))
        singles = ctx.enter_context(tc.tile_pool(name="singles", bufs=1))
        gp = ctx.enter_context(tc.tile_pool(name="gp", bufs=2, space="PSUM"))
        dqp = ctx.enter_context(tc.tile_pool(name="dqp", bufs=3))

        wmb = consts.tile([P, P], BF16)
        nc.vector.memset(wmb, 1.0)
        two_b = consts.tile([P, 1], F32)
        nc.vector.memset(two_b, 2.0)

        # featT: one chunk per slot-pair, ALL on the SP HWDGE ring —
        # measured: the SP ring delivers ~1.5us sooner than the ACT
        # ring, and single-ring FIFO issue (~650ns/chunk descriptor
        # generation) staggers deliveries to exactly the pipeline pace.
        fts = {}
        for c, (s0, s1) in enumerate(pairs):
            w = (s1 - s0) * SLOT
            ftc = singles.tile([P, w], BF16, tag=f"ft{c}")
            nc.sync.dma_start(out=ftc, in_=ft_d[:, s0 * SLOT : s0 * SLOT + w])
            for s in range(s0, s1):
                fts[s] = (s0, ftc)

        def ft_sl(s, a, b):
            s0, ftc = fts[s]
            off = (s - s0) * SLOT
            return ftc[:, off + a : off + b]

        # PE warmup: busy the array from the prologue barrier until the
        # first chunk lands, pushing HAM toward 8/8.
        wm = gp.tile([P, 4 * BANK], F32, tag="gt")
        for _ in range(18):
            nc.tensor.matmul(wm[:, 0:P], wmb, wmb, start=True, stop=True)

        # acc columns [0, nslots) <- acc_a, [nslots, 2*nslots) <- acc_b
        acc = singles.tile([P, 2 * nslots], F16)
        nc.vector.memset(acc[:, nslots : 2 * nslots], 0.0)

        for c, (s0, s1) in enumerate(pairs):
            ns = s1 - s0
            W = widths[s0]
            two = W > P
            gw = 2 * W - P if two else W
            gt = gp.tile([P, 4 * BANK], F32, tag="gt")
            for k in range(ns):
                s = s0 + k
                o = k * BANK
                nc.tensor.matmul(gt[:, o : o + W], ft_sl(s, 0, P),
                                 ft_sl(s, 0, W), start=True, stop=True)
                if two:
                    nc.tensor.matmul(gt[:, o + W : o + gw],
                                     ft_sl(s, P, 2 * P),
                                     ft_sl(s, P, W),
                                     start=True, stop=True)
            dq = dqp.tile([P, ns * gw], BF16, tag="dq")
            nc.scalar.activation(
                dq if ns == 1 else dq[:, :].rearrange("p (n w) -> p n w", w=gw),
                gt[:, 0:gw] if ns == 1 else _ap3(gt, 0, BANK, ns, gw),
                mybir.ActivationFunctionType.Sqrt,
                scale=-2.0, bias=two_b[:, 0:1],
            )
            # fp16 out: DVE still accumulates internally in fp32; the
            # single output rounding (2^-11 rel) is ~1e-4 of the loss.
            with nc.allow_low_precision(reason="fp16 slot sums, fp32 accum"):
                nc.vector.tensor_reduce(
                    acc[:, s0:s1],
                    dq[:, 0:W] if ns == 1 else _ap3(dq, 0, gw, ns, W),
                    axis=mybir.AxisListType.X, op=mybir.AluOpType.add,
                )
                if two:
                    nc.vector.tensor_reduce(
                        acc[:, nslots + s0 : nslots + s1],
                        dq[:, P:gw] if ns == 1 else _ap3(dq, P, gw, ns, gw - P),
                        axis=mybir.AxisListType.X, op=mybir.AluOpType.add,
                    )

        # raw per-partition slot sums out; weighting happens on host
        nc.sync.dma_start(out=out_d[:, :], in_=acc)

    nc.compile()
    return nc


def _shard_inputs(features: np.ndarray, labels: np.ndarray):
    """Sort classes by size, snake-deal to cores, upload scaled
    normalized transposed bf16 features; exact host-side correction."""
    n, d = features.shape
    assert d == P
    classes, counts = np.unique(labels, return_counts=True)
    c = len(classes)
    order_cls = np.argsort(-counts, kind="stable")
    nslots = -(-c // N_CORES)

    deal = [[] for _ in range(N_CORES)]
    for k in range(nslots):
        grp = order_cls[k * N_CORES : (k + 1) * N_CORES]
        seq = range(N_CORES) if k % 2 == 0 else range(N_CORES - 1, -1, -1)
        for core, g in zip(seq, grp):
            deal[core].append(int(g))

    norm = np.linalg.norm(features.astype(np.float32), axis=1, keepdims=True)
    fn = (features * np.float32(SCL) / np.maximum(norm, 1e-12)).astype(
        ml_dtypes.bfloat16)

    argcls = np.argsort(labels, kind="stable")
    bounds = np.concatenate([[0], np.cumsum(counts)])

    # group-uniform widths (max over cores of that rank's class size,
    # then max over the slots of each ACT group)
    widths = []
    for k in range(nslots):
        wk = 1
        for core in range(N_CORES):
            if k < len(deal[core]):
                wk = max(wk, int(counts[deal[core][k]]))
        widths.append(wk)
    for s0, s1 in _groups(nslots):
        wg = max(widths[s0:s1])
        for k in range(s0, s1):
            widths[k] = wg

    # per-row device diagonal value from the uploaded bf16 data
    fn32 = fn.astype(np.float32)
    gdiag = np.einsum("nd,nd->n", fn32, fn32)
    dev_diag = np.sqrt(np.maximum(2.0 - 2.0 * gdiag, 0.0)).astype(np.float64)

    sq2 = float(np.sqrt(np.float32(2.0)))

    in_maps = []
    wrows = []
    junk = 0.0
    for core in range(N_CORES):
        ft = np.zeros((P, nslots * SLOT), ml_dtypes.bfloat16)
        wrow = np.zeros(nslots, np.float64)
        for k in range(nslots):
            W = widths[k]
            if k >= len(deal[core]):
                continue
            g = deal[core][k]
            cnt = int(counts[g])
            rows_g = argcls[bounds[g] : bounds[g + 1]]
            ft[:, k * SLOT : k * SLOT + cnt] = fn[rows_g].T
            w_s = 1.0 / (cnt * n)
            wrow[k] = w_s
            # exact junk: sqrt(2) per zero-pair cell, with the acc_a /
            # acc_b multiplicity; device diagonal mass per valid row.
            vr0 = min(cnt, P)
            # region r0 [128, W]: zero cells = all except valid x valid
            z_r0 = P * W - vr0 * cnt
            # cols >= 128 are double counted (acc_b covers [P, gw))
            if W > P:
                z_r0 += (P * (W - P)) - vr0 * max(0, cnt - P)
                # region r1c1 [128, W-128]
                vr1 = max(0, cnt - P)
                z_r1 = P * (W - P) - vr1 * vr1
            else:
                z_r1 = 0
            junk += w_s * sq2 * (z_r0 + z_r1)
            junk += w_s * float(dev_diag[rows_g].sum())
        in_maps.append({"feat_t": ft})
        wrows.append(wrow)
    return in_maps, widths, junk, wrows


def _run(features, labels, **spmd_kwargs):
    features = np.asarray(features, np.float32)
    labels = np.asarray(labels).reshape(-1)
    in_maps, widths, junk, wrows = _shard_inputs(features, labels)
    nc = _build_program(widths)
    res = run_bass_kernel_spmd(nc, in_maps, core_ids=list(range(N_CORES)),
                               **spmd_kwargs)
    ns = len(widths)
    total = 0.0
    for core, r in enumerate(res.results):
        acc = np.asarray(r["out"], np.float64)       # [P, 2*ns]
        slot_sums = (acc[:, :ns] + acc[:, ns:]).sum(axis=0)
        total += float(slot_sums @ wrows[core])
    return np.float32(total - junk), res


def kernel(features, labels):
    out, _ = _run(features, labels)
    return out
